# revision 1
# baseline (speedup 1.0000x reference)
"""Trainium2 Bass kernel for nn_BasicBlock_1w1a (binary conv BasicBlock).

Self-contained: takes FULL inputs (batch 64), shards batch across 8 NeuronCores,
runs a single SPMD Bass/Tile kernel with in-kernel AllGathers for the
training-mode BatchNorm batch statistics, gathers the full output.

Per block (twice):
  S      = conv3x3(sign(x), sign(w))        # fp8 DoubleRow matmuls, exact
  gate   = sigmoid(BN_dada(avgpool8(x) @ dw))
  u      = prelu(S * alpha * gate, a)       # gate/alpha folded into BN affine
  out    = BN(u) * g + b + x                # batch stats via AllGather

v3 structure:
  - conv matmuls grouped (oi, img-quad, row-half): one LDWEIGHTS feeds 4
    N=512 matmuls; 8x [128,512] psum ring
  - BN statistics via DVE bn_stats on each 512-px eviction (no Square pass)
  - block-2 dada pools from pool(u') and pool(x): p2 = gA1*pool_u + p1
    (the per-channel +B1 shift cancels inside the dada BN), so the dada2
    matmuls/AllGather run inside the stat-AllGather gap / conv2 window
  - mid-block affine: DVE scale + GpSimd add; B1 folded into the next
    sign's ACT bias and the final affine's B1+B2
  - avgpool stage-1 as GpSimd add-trees, stage-2 DVE
  - tail affine split ACT/DVE/GpSimd with immediate per-tile DMA out
  - PE keep-warm fp32 matmul chain spanning the exposed AllGather gap
"""
import os
import sys

sys.path.insert(0, "/opt/trn_rl_repo")

import numpy as np
import ml_dtypes

import concourse.bass as bass
import concourse.bacc as bacc
import concourse.tile as tile
import concourse.mybir as mybir
from concourse import bass_utils

P = 128
CI = 2
NIMG = 8
NCORES = 8
H = W = 32
S = H * W
SP = 34 * 34
EPS = 1e-5
MAGIC = 0x5F3759DF
AF = mybir.ActivationFunctionType
ALU = mybir.AluOpType
DT = mybir.dt
X_AXIS = mybir.AxisListType.X

_CACHE = {}


def _build():
    nc = bacc.Bacc("TRN2", target_bir_lowering=False, debug=False,
                   num_devices=NCORES)

    x_in = nc.dram_tensor("x", [NIMG, 256, S], DT.float32, kind="ExternalInput")
    w1_in = nc.dram_tensor("w1sb", [P, CI, 9, 2, P], DT.float8e4,
                           kind="ExternalInput")
    w2_in = nc.dram_tensor("w2sb", [P, CI, 9, 2, P], DT.float8e4,
                           kind="ExternalInput")
    # dada weights split hi/lo bf16: [c_lo, ci, hilo, oi, o_lo]
    dw1_in = nc.dram_tensor("dwt1", [P, CI, 2, 2, P], DT.bfloat16,
                            kind="ExternalInput")
    dw2_in = nc.dram_tensor("dwt2", [P, CI, 2, 2, P], DT.bfloat16,
                            kind="ExternalInput")
    # packed per-channel params: j = 0:alpha 1:a 2:g 3:b 4:dg 5:db -> [P, 6, CI]
    pk1_in = nc.dram_tensor("pk1", [P, 6, CI], DT.float32, kind="ExternalInput")
    pk2_in = nc.dram_tensor("pk2", [P, 6, CI], DT.float32, kind="ExternalInput")
    out_t = nc.dram_tensor("out", [NIMG, 256, S], DT.float32,
                           kind="ExternalOutput")

    with tile.TileContext(nc) as tc:
        with tc.tile_pool(name="big", bufs=1) as big, \
             tc.tile_pool(name="small", bufs=1) as small, \
             tc.tile_pool(name="psum", bufs=8, space="PSUM") as psum_pool, \
             tc.tile_pool(name="tmp", bufs=4) as tmppool, \
             tc.tile_pool(name="poola", bufs=3) as poola_pool, \
             tc.tile_pool(name="dram", bufs=1, space="DRAM") as dram:

            # ---- warmup collective: absorbs ncfw init + SPMD launch skew ----
            wu = small.tile([P, 1], DT.float32, tag="wu")
            nc.gpsimd.memset(wu[:], 1.0)
            wu_i = dram.tile([P, 1], DT.float32, tag="wu_i")
            wu_o = dram.tile([P * NCORES, 1], DT.float32, tag="wu_o")
            nc.sync.dma_start(wu_i[:], wu[:])
            nc.gpsimd.collective_compute(
                "AllGather", ALU.bypass, replica_groups=[list(range(NCORES))],
                ins=[wu_i[:].opt()], outs=[wu_o[:].opt()])

            def allreduce_stats(stat_sb, out_sb, widx, name):
                """AllGather [128,4] partials + deterministic local reduce."""
                bi = dram.tile([P, 4], DT.float32, tag=f"bi_{name}{widx}")
                bo = dram.tile([P * NCORES, 4], DT.float32,
                               tag=f"bo_{name}{widx}")
                nc.sync.dma_start(bi[:], stat_sb[:])
                nc.gpsimd.collective_compute(
                    "AllGather", ALU.bypass,
                    replica_groups=[list(range(NCORES))],
                    ins=[bi[:].opt()], outs=[bo[:].opt()])
                gath = small.tile([P, NCORES, 4], DT.float32,
                                  tag=f"gth_{name}{widx}")
                nc.sync.dma_start(
                    gath[:], bo[:].rearrange("(r p) c -> p r c", p=P))
                nc.vector.tensor_reduce(out_sb[:],
                                        gath[:].rearrange("p r c -> p c r"),
                                        axis=X_AXIS, op=ALU.add)

            xt = big.tile([P, NIMG, CI, S], DT.float32, tag="xt")
            ut = big.tile([P, 2, NIMG, S], DT.float32, tag="ut")
            # sign pads split per image-pair: keeps the scheduler's fused
            # dependency waits fine-grained (one tile would make the first
            # conv group wait on every sign write)
            spads = [big.tile([P, CI, 2, SP], DT.float8e4, tag=f"spad{q}",
                              name=f"spad{q}") for q in range(4)]
            w1sb = big.tile([P, CI, 9, 2, P], DT.float8e4, tag="w1")
            w2sb = big.tile([P, CI, 9, 2, P], DT.float8e4, tag="w2")
            dwt1 = big.tile([P, CI, 2, 2, P], DT.bfloat16, tag="dwt1")
            dwt2 = big.tile([P, CI, 2, 2, P], DT.bfloat16, tag="dwt2")
            pk1 = big.tile([P, 6, CI], DT.float32, tag="pk1")
            pk2 = big.tile([P, 6, CI], DT.float32, tag="pk2")
            # per-(img,half) BN partials from bn_stats: [oi, n, 4 grp, (c,m,M2)]
            bnst = {
                1: small.tile([P, 2, NIMG, 12], DT.float32, tag="bnst1",
                              name="bnst1"),
                2: small.tile([P, 2, NIMG, 12], DT.float32, tag="bnst2",
                              name="bnst2"),
            }
            # pool sums of u' (block1) for the dada2-pools identity
            pools_u = big.tile([P, 2, NIMG, 16], DT.float32, tag="pu")

            for q in range(4):
                nc.gpsimd.memset(
                    spads[q][:].rearrange("p c n s -> p (c n s)")
                    .bitcast(DT.int32), 0)

            def dma_x(n):
                # single hwdge ring: x-in is transfer-rate bound either way;
                # keep ACT/GpSimd queues free
                xv = x_in[n].rearrange("(ci p) s -> p ci s", p=P)
                nc.sync.dma_start(xt[:, n, 0, :], xv[:, 0, :])
                nc.sync.dma_start(xt[:, n, 1, :], xv[:, 1, :])

            for n in range(4):
                dma_x(n)
            nc.sync.dma_start(w1sb[:], w1_in[:])
            nc.sync.dma_start(pk1[:], pk1_in[:])
            nc.sync.dma_start(dwt1[:], dw1_in[:])
            for n in range(4, NIMG):
                dma_x(n)
            nc.sync.dma_start(w2sb[:], w2_in[:])
            nc.sync.dma_start(dwt2[:], dw2_in[:])
            nc.sync.dma_start(pk2[:], pk2_in[:])

            def sign_into_spad(n, ci, bias=0.0):
                view = spads[n >> 1][:, ci, n & 1, :].rearrange(
                    "p (r c) -> p r c", r=34)
                nc.scalar.activation(
                    view[:, 1:33, 1:33],
                    xt[:, n, ci, :].rearrange("p (h w) -> p h w", h=H),
                    AF.Sign, bias=bias)

            def pool_dve(src_ap, dst_16, key):
                """8x8 sum-pool of one [P, 1024] (h,w) plane, DVE 2-stage."""
                pa = poola_pool.tile([P, H * 4], DT.float32, tag="poola",
                                     name=f"pa_{key}")
                nc.vector.tensor_reduce(
                    pa[:],
                    src_ap.rearrange("p (h pw w) -> p h pw w", h=H, pw=4),
                    axis=X_AXIS, op=ALU.add)
                nc.vector.tensor_reduce(
                    dst_16.rearrange("p (ph pw) -> p ph pw", ph=4),
                    pa[:].rearrange("p (ph hh pw) -> p ph pw hh", ph=4, hh=8),
                    axis=X_AXIS, op=ALU.add)

            def pool_tree(src_ap, dst_16, key):
                """Same pool, stage-1 as 3 GpSimd add-tree ops (off-DVE)."""
                t1 = poola_pool.tile([P, 512], DT.float32, tag="poolt1",
                                     name=f"pt1_{key}")
                t2 = poola_pool.tile([P, 256], DT.float32, tag="poolt2",
                                     name=f"pt2_{key}")
                pa = poola_pool.tile([P, H * 4], DT.float32, tag="poola",
                                     name=f"pa_{key}")
                xv = src_ap.rearrange("p (h pw a b) -> p h pw a b", h=H,
                                      pw=4, a=2)
                nc.gpsimd.tensor_add(
                    t1[:].rearrange("p (h pw b) -> p h pw b", h=H, pw=4),
                    xv[:, :, :, 0, :], xv[:, :, :, 1, :])
                t1v = t1[:].rearrange("p (h pw a b) -> p h pw a b", h=H,
                                      pw=4, a=2)
                nc.gpsimd.tensor_add(
                    t2[:].rearrange("p (h pw b) -> p h pw b", h=H, pw=4),
                    t1v[:, :, :, 0, :], t1v[:, :, :, 1, :])
                t2v = t2[:].rearrange("p (h pw a) -> p h pw a", h=H, pw=4)
                nc.gpsimd.tensor_add(
                    pa[:].rearrange("p (h pw) -> p h pw", h=H),
                    t2v[:, :, :, 0], t2v[:, :, :, 1])
                nc.vector.tensor_reduce(
                    dst_16.rearrange("p (ph pw) -> p ph pw", ph=4),
                    pa[:].rearrange("p (ph hh pw) -> p ph pw hh", ph=4, hh=8),
                    axis=X_AXIS, op=ALU.add)

            def rsqrt_inplace(k, t, e1):
                """k = 1/sqrt(t), all DVE (quake seed + 3 Newton)."""
                ki = k.bitcast(DT.int32)
                nc.vector.tensor_scalar(ki, t.bitcast(DT.int32), 1, None,
                                        ALU.arith_shift_right)
                nc.vector.tensor_scalar(ki, ki, MAGIC, None, ALU.subtract)
                nc.vector.tensor_scalar(ki, ki, -1, None, ALU.mult)
                for _ in range(3):
                    nc.vector.tensor_mul(e1, k, k)
                    nc.vector.tensor_mul(e1, e1, t)
                    nc.vector.tensor_scalar(e1, e1, -0.5, 1.5, ALU.mult,
                                            ALU.add)
                    nc.vector.tensor_mul(k, k, e1)

            p_tiles = {
                1: small.tile([P, CI, NIMG, 16], DT.float32, name="p_t1",
                              tag="p1"),
                2: small.tile([P, CI, NIMG, 16], DT.float32, name="p_t2",
                              tag="p2"),
            }

            def conv_quad(widx, wsb, pk, oi, imgs, half):
                """one LDW per kk feeds len(imgs) N=512 DoubleRow matmuls."""
                tl = {n: psum_pool.tile([P, 512], DT.float32, tag="ps",
                                        name=f"ps{widx}_{oi}_{half}_{n}")
                      for n in imgs}
                for kk in range(9):
                    dy, dx = divmod(kk, 3)
                    lhsT = wsb[:, :, kk, oi, :]
                    for j, n in enumerate(imgs):
                        sview = spads[n >> 1][:, :, n & 1, :].rearrange(
                            "p ci (r c) -> p ci r c", r=34)
                        mm = nc.tensor.matmul(
                            tl[n][:], lhsT,
                            sview[:, :, half * 16 + dy:half * 16 + dy + 16,
                                  dx:dx + 32],
                            start=(kk == 0), stop=(kk == 8),
                            perf_mode=mybir.MatmulPerfMode.DoubleRow)
                        if j > 0:
                            # same stationary weights as the j==0 matmul of
                            # this kk — skip the redundant LDWEIGHTS
                            mm.ins.ldweights = False
                for n in imgs:
                    u_sl = ut[:, oi, n, half * 512:(half + 1) * 512]
                    nc.scalar.activation(u_sl, tl[n][:], AF.Prelu,
                                         alpha=pk[:, 1, oi:oi + 1])
                    nc.vector.bn_stats(
                        bnst[widx][:, oi, n, half * 6:(half + 1) * 6], u_sl)
                    if widx == 1 and half == 1:
                        # keep the GpSimd queue short (it carries the
                        # collective triggers): oi=0 pools on DVE
                        pool_fn = pool_dve if oi == 0 else pool_tree
                        pool_fn(ut[:, oi, n, :], pools_u[:, oi, n, :],
                                f"u_{oi}_{n}")

            def dada_mms(widx, dwt, p_t):
                """hi/lo split + 16 dada matmuls + psum evict -> ysb."""
                ph = small.tile([P, CI, NIMG * 16], DT.bfloat16, tag=f"ph{widx}")
                pl = small.tile([P, CI, NIMG * 16], DT.bfloat16, tag=f"pl{widx}")
                ysb = small.tile([P, 2, NIMG * 16], DT.float32, tag=f"y{widx}")
                # on GpSimd: the DVE queue is deep in bn_stats/pools here and
                # a late hi/lo stalls the dada matmuls in the PE FIFO
                nc.gpsimd.tensor_copy(ph[:],
                                      p_t[:].rearrange("p c n s -> p c (n s)"))
                nc.gpsimd.tensor_sub(pl[:],
                                     p_t[:].rearrange("p c n s -> p c (n s)"),
                                     ph[:])
                for oi in range(2):
                    psy = psum_pool.tile([P, NIMG * 16], DT.float32,
                                         tag="ps", name=f"psy{widx}_{oi}")
                    terms = [(hl, pp) for hl in range(2) for pp in (ph, pl)]
                    for ci in range(CI):
                        for ti, (hl, pp) in enumerate(terms):
                            nc.tensor.matmul(
                                psy[:], dwt[:, ci, hl, oi, :], pp[:, ci, :],
                                start=(ci == 0 and ti == 0),
                                stop=(ci == CI - 1 and ti == len(terms) - 1))
                    nc.scalar.activation(ysb[:, oi, :], psy[:], AF.Copy)
                return ysb

            def dada_stats(widx, pk, ysb, gate, ystat, ar_y):
                """BN-dada stats from ysb -> AG -> gate."""
                ynst = small.tile([P, 2, 6], DT.float32, tag=f"yn{widx}")
                m_s = small.tile([P, 2, NIMG], DT.float32, tag=f"ms{widx}")
                msq = small.tile([P, 2, 2], DT.float32, tag=f"msq{widx}")
                for oi in range(2):
                    nc.vector.bn_stats(ynst[:, oi, :], ysb[:, oi, :])
                nc.vector.tensor_reduce(
                    m_s[:], ysb[:].rearrange("p c (n q) -> p c n q", n=NIMG),
                    axis=X_AXIS, op=ALU.add)
                yv = ynst[:].rearrange("p c (g f) -> p c g f", g=2)
                # ysum = 64*(m_e + m_o); ysq = M2_e + M2_o + 64*(m_e^2+m_o^2)
                nc.vector.tensor_reduce(ystat[:, 0:2], yv[:, :, :, 1],
                                        axis=X_AXIS, op=ALU.add)
                nc.vector.tensor_scalar(ystat[:, 0:2], ystat[:, 0:2], 64.0,
                                        None, ALU.mult)
                nc.vector.tensor_mul(msq[:], yv[:, :, :, 1], yv[:, :, :, 1])
                nc.vector.tensor_reduce(ystat[:, 2:4], msq[:], axis=X_AXIS,
                                        op=ALU.add)
                nc.vector.tensor_scalar(ystat[:, 2:4], ystat[:, 2:4], 64.0,
                                        None, ALU.mult)
                m2s = small.tile([P, 2], DT.float32, tag=f"m2s{widx}")
                nc.vector.tensor_reduce(m2s[:], yv[:, :, :, 2], axis=X_AXIS,
                                        op=ALU.add)
                nc.vector.tensor_add(ystat[:, 2:4], ystat[:, 2:4], m2s[:])

                allreduce_stats(ystat, ar_y, widx, "y")

                cnt_y = float(NCORES * NIMG * 16)
                for oi in range(2):
                    t = small.tile([P, 1], DT.float32, tag=f"t{widx}_{oi}")
                    mu = small.tile([P, 1], DT.float32, tag=f"mu{widx}_{oi}")
                    k = small.tile([P, 1], DT.float32, tag=f"k{widx}_{oi}")
                    e1 = small.tile([P, 1], DT.float32, tag=f"e{widx}_{oi}")
                    A16 = small.tile([P, 1], DT.float32, tag=f"A{widx}_{oi}")
                    B = small.tile([P, 1], DT.float32, tag=f"B{widx}_{oi}")
                    nc.vector.tensor_scalar(t[:], ar_y[:, 2 + oi:3 + oi],
                                            1.0 / cnt_y, EPS, ALU.mult, ALU.add)
                    nc.vector.tensor_scalar(mu[:], ar_y[:, oi:oi + 1],
                                            1.0 / cnt_y, None, ALU.mult)
                    nc.vector.tensor_mul(e1[:], mu[:], mu[:])
                    nc.vector.tensor_sub(t[:], t[:], e1[:])
                    rsqrt_inplace(k[:], t[:], e1[:])
                    # A = k*dg; sigmoid(A*(m_s/16) + B): scale = A/16
                    nc.vector.tensor_mul(A16[:], k[:], pk[:, 4, oi:oi + 1])
                    nc.vector.tensor_mul(B[:], mu[:], A16[:])
                    nc.vector.tensor_sub(B[:], pk[:, 5, oi:oi + 1], B[:])
                    nc.vector.tensor_scalar(A16[:], A16[:], 1.0 / 16.0, None,
                                            ALU.mult)
                    sig = small.tile([P, NIMG], DT.float32,
                                     tag=f"sg{widx}_{oi}")
                    nc.scalar.activation(sig[:], m_s[:, oi, :], AF.Sigmoid,
                                         bias=B[:], scale=A16[:])
                    nc.vector.tensor_scalar(gate[:, oi, :], sig[:],
                                            pk[:, 0, oi:oi + 1], None, ALU.mult)

            def main_stats(widx, gate, ustat):
                """usum/usq per image from bn_stats partials, gate-weighted."""
                bv = bnst[widx][:].rearrange("p c n (g f) -> p c n g f", g=4)
                ms = small.tile([P, 2, NIMG], DT.float32, tag=f"us_m{widx}")
                mq = small.tile([P, 2, NIMG, 4], DT.float32, tag=f"us_q{widx}")
                qs = small.tile([P, 2, NIMG], DT.float32, tag=f"us_s{widx}")
                m2 = small.tile([P, 2, NIMG], DT.float32, tag=f"us_2{widx}")
                w8 = small.tile([P, 2, NIMG], DT.float32, tag=f"us_w{widx}")
                g2 = small.tile([P, 2, NIMG], DT.float32, tag=f"us_g{widx}")
                # sum(u) per (oi,n) = 256 * sum of 4 group means
                nc.vector.tensor_reduce(ms[:], bv[:, :, :, :, 1], axis=X_AXIS,
                                        op=ALU.add)
                # sum(u^2) = sum M2 + 256 * sum m^2
                nc.vector.tensor_mul(mq[:], bv[:, :, :, :, 1],
                                     bv[:, :, :, :, 1])
                nc.vector.tensor_reduce(qs[:], mq[:], axis=X_AXIS, op=ALU.add)
                nc.vector.tensor_reduce(m2[:], bv[:, :, :, :, 2], axis=X_AXIS,
                                        op=ALU.add)
                nc.vector.tensor_scalar(qs[:], qs[:], 256.0, None, ALU.mult)
                nc.vector.tensor_add(qs[:], qs[:], m2[:])
                # gate-weighted: sum_n g*usum, sum_n g^2*usq   (256 into scale)
                nc.vector.tensor_mul(w8[:], ms[:], gate[:])
                nc.vector.tensor_reduce(ustat[:, 0:2], w8[:], axis=X_AXIS,
                                        op=ALU.add)
                nc.vector.tensor_scalar(ustat[:, 0:2], ustat[:, 0:2], 256.0,
                                        None, ALU.mult)
                nc.vector.tensor_mul(g2[:], gate[:], gate[:])
                nc.vector.tensor_mul(w8[:], qs[:], g2[:])
                nc.vector.tensor_reduce(ustat[:, 2:4], w8[:], axis=X_AXIS,
                                        op=ALU.add)

            def bn_affine(widx, pk, ar_u, gate, AB, gA):
                """A = k*g, B = b - A*mu, gA[n] = A*gate[n]."""
                cnt_u = float(NCORES * NIMG * S)
                for ci in range(2):
                    t = small.tile([P, 1], DT.float32, tag=f"tu{widx}_{ci}")
                    mu = small.tile([P, 1], DT.float32, tag=f"muu{widx}_{ci}")
                    k = small.tile([P, 1], DT.float32, tag=f"ku{widx}_{ci}")
                    e1 = small.tile([P, 1], DT.float32, tag=f"eu{widx}_{ci}")
                    nc.vector.tensor_scalar(t[:], ar_u[:, 2 + ci:3 + ci],
                                            1.0 / cnt_u, EPS, ALU.mult, ALU.add)
                    nc.vector.tensor_scalar(mu[:], ar_u[:, ci:ci + 1],
                                            1.0 / cnt_u, None, ALU.mult)
                    nc.vector.tensor_mul(e1[:], mu[:], mu[:])
                    nc.vector.tensor_sub(t[:], t[:], e1[:])
                    rsqrt_inplace(k[:], t[:], e1[:])
                    nc.vector.tensor_mul(AB[:, 0, ci:ci + 1], k[:],
                                         pk[:, 2, ci:ci + 1])
                    nc.vector.tensor_mul(e1[:], mu[:], AB[:, 0, ci:ci + 1])
                    nc.vector.tensor_sub(AB[:, 1, ci:ci + 1],
                                         pk[:, 3, ci:ci + 1], e1[:])
                for ci in range(2):
                    nc.vector.tensor_scalar(gA[:, ci, :], gate[:, ci, :],
                                            AB[:, 0, ci:ci + 1], None, ALU.mult)

            gate1 = small.tile([P, 2, NIMG], DT.float32, tag="g1")
            gate2 = small.tile([P, 2, NIMG], DT.float32, tag="g2")
            ystat1 = small.tile([P, 4], DT.float32, tag="ys1")
            ystat2 = small.tile([P, 4], DT.float32, tag="ys2")
            ar_y1 = small.tile([P, 4], DT.float32, tag="ary1")
            ar_y2 = small.tile([P, 4], DT.float32, tag="ary2")
            ustat1 = small.tile([P, 4], DT.float32, tag="us1")
            ustat2 = small.tile([P, 4], DT.float32, tag="us2")
            ar_u1 = small.tile([P, 4], DT.float32, tag="aru1")
            ar_u2 = small.tile([P, 4], DT.float32, tag="aru2")
            AB1 = small.tile([P, 2, 2], DT.float32, tag="ab1")
            AB2 = small.tile([P, 2, 2], DT.float32, tag="ab2")
            gA1 = small.tile([P, 2, NIMG], DT.float32, tag="ga1")
            gA2 = small.tile([P, 2, NIMG], DT.float32, tag="ga2")
            Bp = small.tile([P, 2], DT.float32, tag="bp")

            QUADS = [(0, 1, 2, 3), (4, 5, 6, 7)]

            # ================= block 1 =================
            # signs/pools interleaved with matmul groups: small first groups
            # (the group's fused wait is conservative — keep early writers few)
            for grp in [(0, 1), (2, 3), (4, 5, 6, 7)]:
                for n in grp:
                    for ci in range(CI):
                        sign_into_spad(n, ci)
                        pool_dve(xt[:, n, ci, :], p_tiles[1][:, ci, n, :],
                                 f"x1_{n}_{ci}")
                for half in range(2):
                    conv_quad(1, w1sb, pk1, 0, grp, half)
            # pools1 are done by here; dada matmuls slot into the PE stream,
            # stats/AG fire mid-conv1 while the oi=1 groups stream
            ysb1 = dada_mms(1, dwt1, p_tiles[1])
            dada_stats(1, pk1, ysb1, gate1, ystat1, ar_y1)
            for imgs in QUADS:
                for half in range(2):
                    conv_quad(1, w1sb, pk1, 1, imgs, half)

            main_stats(1, gate1, ustat1)
            allreduce_stats(ustat1, ar_u1, 1, "u")

            # keep-warm chain: fp32 MMs reading ut (ready at conv1 end),
            # spans the AllGather gap so conv2 starts at K=8/8
            pd = psum_pool.tile([P, 512], DT.float32, tag="ps", name="pd_warm")
            for i in range(8):
                nc.tensor.matmul(pd[:], ut[:, 0, 0, 0:P],
                                 ut[:, 1, 7, 512:1024],
                                 start=(i == 0), stop=(i == 7))

            bn_affine(1, pk1, ar_u1, gate1, AB1, gA1)

            # dada2 pools via identity: p2 = gA1*pool(u') + pool(x)
            # (+64*B1 shift per channel cancels inside the dada BN)
            for ci in range(CI):
                for n in range(NIMG):
                    nc.vector.scalar_tensor_tensor(
                        p_tiles[2][:, ci, n, :], pools_u[:, ci, n, :],
                        gA1[:, ci, n:n + 1], p_tiles[1][:, ci, n, :],
                        ALU.mult, ALU.add)
            ysb2 = dada_mms(2, dwt2, p_tiles[2])
            dada_stats(2, pk2, ysb2, gate2, ystat2, ar_y2)

            # x1 = gA1[n]*u' + x (B1 folded into sign bias / final affine)
            # scale mostly on ACT (fastest elementwise), adds DVE/GpSimd
            for n in range(NIMG):
                for ci in range(CI):
                    idx = n * 2 + ci
                    tmp = tmppool.tile([P, S], DT.float32, tag="tmp",
                                       name=f"tmid_{n}_{ci}")
                    if idx % 8 < 5:
                        nc.scalar.activation(tmp[:], ut[:, ci, n, :],
                                             AF.Identity,
                                             scale=gA1[:, ci, n:n + 1])
                        eng = nc.gpsimd
                    else:
                        nc.vector.tensor_scalar(tmp[:], ut[:, ci, n, :],
                                                gA1[:, ci, n:n + 1], None,
                                                ALU.mult)
                        eng = nc.vector
                    eng.tensor_add(xt[:, n, ci, :], tmp[:], xt[:, n, ci, :])
                    sign_into_spad(n, ci, bias=AB1[:, 1, ci:ci + 1])

            # ================= block 2 =================
            for oi in range(2):
                for imgs in QUADS:
                    for half in range(2):
                        conv_quad(2, w2sb, pk2, oi, imgs, half)

            main_stats(2, gate2, ustat2)
            allreduce_stats(ustat2, ar_u2, 2, "u")
            bn_affine(2, pk2, ar_u2, gate2, AB2, gA2)
            # B' = B1 + B2 (skip path carries the un-shifted v = x1 - B1)
            nc.vector.tensor_add(Bp[:], AB1[:, 1, :], AB2[:, 1, :])

            # out = gA2[n]*u' + B' + v ; scales mostly ACT, adds DVE/GpSimd,
            # out-DMA on two issue queues
            for n in range(NIMG):
                for ci in range(CI):
                    idx = n * 2 + ci
                    ov = out_t[n].rearrange("(ci p) s -> p ci s", p=P)
                    tmp = tmppool.tile([P, S], DT.float32, tag="tmp",
                                       name=f"tout_{n}_{ci}")
                    if idx % 4 < 3:
                        nc.scalar.activation(tmp[:], ut[:, ci, n, :],
                                             AF.Identity,
                                             bias=Bp[:, ci:ci + 1],
                                             scale=gA2[:, ci, n:n + 1])
                    else:
                        nc.vector.tensor_scalar(tmp[:], ut[:, ci, n, :],
                                                gA2[:, ci, n:n + 1],
                                                Bp[:, ci:ci + 1],
                                                ALU.mult, ALU.add)
                    eng = nc.gpsimd if idx % 2 == 0 else nc.vector
                    eng.tensor_add(xt[:, n, ci, :], tmp[:], xt[:, n, ci, :])
                    deng = nc.sync if ci == 0 else nc.scalar
                    deng.dma_start(ov[:, ci, :], xt[:, n, ci, :])

    nc.compile()
    return nc


def _pack_w(w):
    ws = np.sign(w.astype(np.float32))
    t = ws.reshape(2, P, CI, P, 3, 3)           # oi, o_lo, ci, c_lo, dy, dx
    t = t.transpose(3, 2, 4, 5, 0, 1)           # c_lo, ci, dy, dx, oi, o_lo
    return np.ascontiguousarray(t.reshape(P, CI, 9, 2, P)).astype(
        ml_dtypes.float8_e4m3)


def _pack_dw(dw):
    d = (dw.astype(np.float32) / 64.0).reshape(2, P, CI, P)  # oi,o_lo,ci,c_lo
    d = d.transpose(3, 2, 0, 1)                               # c_lo,ci,oi,o_lo
    hi = d.astype(ml_dtypes.bfloat16)
    lo = (d - hi.astype(np.float32)).astype(ml_dtypes.bfloat16)
    out = np.empty((P, CI, 2, 2, P), ml_dtypes.bfloat16)
    out[:, :, 0] = hi
    out[:, :, 1] = lo
    return out


def _pack_pk(w, a, g, b, dg, db):
    alpha = np.abs(w.astype(np.float32)).mean(axis=(1, 2, 3))
    fields = [alpha, a, g, b, dg, db]
    pk = np.empty((P, 6, CI), np.float32)
    for j, f in enumerate(fields):
        pk[:, j, :] = np.asarray(f, np.float32).reshape(CI, P).T
    return pk


def kernel(**inputs):
    if "nc" not in _CACHE:
        _CACHE["nc"] = _build()
    nc = _CACHE["nc"]

    x = np.asarray(inputs["x"], np.float32).reshape(64, 256, S)
    feed = {
        "w1sb": _pack_w(np.asarray(inputs["w1"])),
        "w2sb": _pack_w(np.asarray(inputs["w2"])),
        "dwt1": _pack_dw(np.asarray(inputs["dw1"])),
        "dwt2": _pack_dw(np.asarray(inputs["dw2"])),
        "pk1": _pack_pk(np.asarray(inputs["w1"]), inputs["a1"], inputs["g1"],
                        inputs["b1"], inputs["dg1"], inputs["db1"]),
        "pk2": _pack_pk(np.asarray(inputs["w2"]), inputs["a2"], inputs["g2"],
                        inputs["b2"], inputs["dg2"], inputs["db2"]),
    }
    in_maps = []
    for c in range(NCORES):
        m = dict(feed)
        m["x"] = np.ascontiguousarray(x[c * NIMG:(c + 1) * NIMG])
        in_maps.append(m)

    trace = bool(int(os.environ.get("BASS_KERNEL_TRACE", "0")))
    res = bass_utils.run_bass_kernel_spmd(
        nc, in_maps, core_ids=list(range(NCORES)), trace=trace)
    kernel.last_results = res

    out = np.concatenate([res.results[c]["out"] for c in range(NCORES)], axis=0)
    return out.reshape(64, 256, H, W)

